# revision 1
# baseline (speedup 1.0000x reference)
"""Causal self-attention on 8 trn2 NeuronCores.

Sharding: core c -> (batch b = c // 4, head-group g = c % 4). Each core
computes 4 of the 16 heads for one batch element and the corresponding
slice of the output projection; the host sums the 4 partial projections
per batch and adds the constant bias terms (bv @ Wp.T + bp) exactly.

All transposes (x.T and the weight slices) are done on the host so the
device only runs natural-layout matmuls. Matmuls run as float32r
(full-rate fp32 PE mode); softmax runs unnormalized with the causal mask
applied additively in PSUM, and the 1/rowsum normalization is folded
into the PSUM eviction of the attention output.
"""

import numpy as np

import concourse.bass as bass
import concourse.mybir as mybir
import concourse.tile as tile
from concourse.bass_utils import run_bass_kernel_spmd

B = 2
T = 2048
C = 1024
H = 16
DH = 64
NCORES = 8
GROUPS = 4           # head groups (tensor parallel)
HPG = H // GROUPS    # heads per group = 4
DG = HPG * DH        # head-group width = 256
CHUNK = 512          # query-block size
NCHUNK = T // CHUNK  # 4
KTILE = 128          # key-block size (PE contraction tile)
F32 = mybir.dt.float32
F32R = mybir.dt.float32r
BF16 = mybir.dt.bfloat16
MASK_NEG = -1e30


def _patch_tile_drain():
    """This walrus build lowers Drain/NOP to a CTRL with a single sync-wait
    slot; TileContext's kernel-tail drain accumulates one wait per live
    semaphore and fails codegen. Split the waits across single-wait NOPs."""
    import bass_rust
    from concourse.tile import TileContext

    def _drain_and_barrier_split(self, tick_clock, wait_clock):
        probe = self.nc.sync.nop()
        wait_clock.add_sem_waits(
            probe.ins, tile.ScopedClock({None: tick_clock.global_clock})
        )
        waits = list(probe.ins.sync_info.on_wait or [])
        probe.ins.sync_info.on_wait = []
        # distribute the final-value waits across engines; the all-engine
        # barrier below joins them before the semaphore reset
        engines = [self.nc.sync, self.nc.tensor, self.nc.vector,
                   self.nc.scalar, self.nc.gpsimd]
        for i, w in enumerate(waits):
            n = engines[i % len(engines)].nop()
            if n.ins.sync_info is None:
                n.ins.sync_info = bass_rust.SyncInfo(on_wait=[w], on_update=[])
            else:
                n.ins.sync_info.on_wait = [w]
        self.nc.sync.drain()
        self.nc.all_engine_barrier()
        assert self.sems is not None
        popped = self.nc._tile_sem_poison_stack.pop()
        assert popped is self._sem_poison
        self.nc.clear_and_free_semaphores(list(self.sems.allocated().values()))
        self.nc.all_engine_barrier()

    TileContext._drain_and_barrier = _drain_and_barrier_split

    # Same single-wait limit applies to every lowered TPB instruction (the
    # 64B formats carry one EVENTS field). Post-process the BIR JSON before
    # walrus: hoist extra semaphore waits onto same-engine NoOps.
    import json as _json

    import concourse.bass2jax as bass2jax
    import concourse.bass_utils as bass_utils

    if getattr(bass_utils.compile_bir_kernel, "_wait_split", False):
        return

    _orig_compile = bass_utils.compile_bir_kernel

    def _split_multi_waits(bir_json):
        m = _json.loads(bir_json)
        counter = 0
        changed = False
        for fn in m["functions"]:
            for blk in fn["blocks"]:
                new_insts = []
                for inst in blk["instructions"]:
                    si = inst.get("sync_info")
                    waits = (si or {}).get("on_wait") or []
                    sem_waits = [w for w in waits if w.get("sync_type") == "semaphore"]
                    if len(waits) > 1 and len(sem_waits) == len(waits):
                        changed = True
                        for w in waits[:-1]:
                            counter += 1
                            new_insts.append({
                                "name": f"I-wsplit{counter}",
                                "opcode": "NoOp",
                                "engine": inst["engine"],
                                "ins": [],
                                "outs": [],
                                "sync_info": {"on_wait": [w], "on_update": []},
                            })
                        si["on_wait"] = [waits[-1]]
                    new_insts.append(inst)
                blk["instructions"] = new_insts
        if not changed:
            return bir_json
        return _json.dumps(m).encode()

    def _compile_bir_kernel_split(bir_json, tmpdir, neff_name="file.neff"):
        return _orig_compile(_split_multi_waits(bir_json), tmpdir, neff_name=neff_name)

    _compile_bir_kernel_split._wait_split = True
    bass_utils.compile_bir_kernel = _compile_bir_kernel_split
    bass2jax.compile_bir_kernel = _compile_bir_kernel_split


def build_kernel():
    _patch_tile_drain()
    nc = bass.Bass(target_bir_lowering=False, trn_type="TRN2")

    xT = nc.dram_tensor("xT", [C, T], F32R, kind="ExternalInput")
    wq = nc.dram_tensor("wq", [C, DG], F32R, kind="ExternalInput")
    wk = nc.dram_tensor("wk", [C, DG], F32R, kind="ExternalInput")
    wv = nc.dram_tensor("wv", [C, DG], F32R, kind="ExternalInput")
    wp = nc.dram_tensor("wp", [DG, C], F32R, kind="ExternalInput")
    bq = nc.dram_tensor("bq", [DG], F32, kind="ExternalInput")
    bk = nc.dram_tensor("bk", [DG], F32, kind="ExternalInput")
    out = nc.dram_tensor("out", [T, C], F32, kind="ExternalOutput")

    KO = C // 128            # 8 contraction subtiles for the projections
    MT = DG // 128           # 2 partition tiles for qT/kT and wp rows
    scale = 1.0 / np.sqrt(DH)

    from contextlib import ExitStack

    with tile.TileContext(nc) as tc, ExitStack() as ctx:
        from concourse.masks import make_identity

        const = ctx.enter_context(tc.tile_pool(name="const", bufs=1))
        xt_pool = ctx.enter_context(tc.tile_pool(name="xt", bufs=2))
        persist = ctx.enter_context(tc.tile_pool(name="persist", bufs=1))
        expst_pool = ctx.enter_context(tc.tile_pool(name="expst", bufs=4))
        small = ctx.enter_context(tc.tile_pool(name="small", bufs=4))
        out_pool = ctx.enter_context(tc.tile_pool(name="outp", bufs=3))
        ps_mm = ctx.enter_context(tc.tile_pool(name="psmm", bufs=2, space="PSUM"))
        ps_s = ctx.enter_context(tc.tile_pool(name="pss", bufs=2, space="PSUM"))
        ps_y = ctx.enter_context(tc.tile_pool(name="psy", bufs=2, space="PSUM"))
        dram_pool = ctx.enter_context(tc.tile_pool(name="dram", bufs=2, space="DRAM"))
        onebuf = ctx.enter_context(tc.tile_pool(name="onebuf", bufs=1))

        # ---- constants ----
        xT_r = xT.rearrange("(ko p) t -> p ko t", p=128)
        out_r = out.rearrange("(tt p) c -> tt p c", p=128)
        _xt_tiles = {}

        def prefetch_xt(n):
            if n not in _xt_tiles:
                xt = xt_pool.tile([128, KO, CHUNK], F32R, tag="xt", name=f"xt{n}")
                for ko in range(KO):
                    nc.sync.dma_start(
                        xt[:, ko, :], xT_r[:, ko, n * CHUNK:(n + 1) * CHUNK]
                    )
                _xt_tiles[n] = xt

        def load_xt(n):
            prefetch_xt(n)
            return _xt_tiles.pop(n)

        wq_sb = const.tile([128, KO, DG], F32R)
        nc.sync.dma_start(wq_sb[:], wq.rearrange("(ko p) d -> p ko d", p=128))
        wk_sb = const.tile([128, KO, DG], F32R)
        nc.sync.dma_start(wk_sb[:], wk.rearrange("(ko p) d -> p ko d", p=128))
        prefetch_xt(0)
        wv_sb = const.tile([128, KO, DG], F32R)
        nc.sync.dma_start(wv_sb[:], wv.rearrange("(ko p) d -> p ko d", p=128))
        bq_sb = const.tile([128, MT], F32)
        nc.sync.dma_start(bq_sb[:], bq.rearrange("(mt p) -> p mt", p=128))
        bk_sb = const.tile([128, MT], F32)
        nc.sync.dma_start(bk_sb[:], bk.rearrange("(mt p) -> p mt", p=128))

        ones_f32 = const.tile([128, 64], F32)
        nc.vector.memset(ones_f32[:], 1.0)
        ones_sb = const.tile([128, 64], F32R)
        nc.vector.tensor_copy(ones_sb[:], ones_f32[:])
        ident = const.tile([128, 128], BF16)
        make_identity(nc, ident)
        # wmask[k, j] = MASK_NEG where j < k + 256, else 0.
        # wmask[:, 256:384] is a strict lower-triangle mask; wmask[:, 128:384]
        # additionally blankets 128 fully-masked columns (used on the last
        # diagonal block so its matmuls can run at N=256 instead of N=128).
        wmask = const.tile([128, 384], BF16)
        nc.gpsimd.memset(wmask[:], 0.0)
        nc.gpsimd.affine_select(
            out=wmask[:],
            in_=wmask[:],
            compare_op=mybir.AluOpType.is_ge,
            fill=MASK_NEG,
            base=-256,
            pattern=[[1, 384]],
            channel_multiplier=-1,
        )

        # ---- persistent activations ----
        qT_sb = persist.tile([128, MT, T], F32R)     # [d_local, T] for 4 heads
        kT_sb = persist.tile([128, MT, T], F32R)
        # [tk_in, tk_tile, h, dh+1]; the last column of each head is a ones
        # column so attn@v also accumulates the softmax denominator l.
        v_sb = persist.tile([128, T // 128, HPG, DH + 1], F32R)
        nc.vector.tensor_copy(
            v_sb[:, :, :, DH].rearrange("p t h -> p (t h)"), ones_sb[:]
        )

        def proj(n):
            cols = slice(n * CHUNK, (n + 1) * CHUNK)
            xt = load_xt(n)

            for w_sb, b_sb, dst in ((wq_sb, bq_sb, qT_sb), (wk_sb, bk_sb, kT_sb)):
                for mt in range(MT):
                    ps = ps_mm.tile([128, CHUNK], F32, tag="mm", name=f"pj{n}_{mt}")
                    for ko in range(KO):
                        nc.tensor.matmul(
                            ps[:],
                            lhsT=w_sb[:, ko, mt * 128:(mt + 1) * 128],
                            rhs=xt[:, ko, :],
                            start=(ko == 0),
                            stop=(ko == KO - 1),
                        )
                    nc.vector.tensor_scalar_add(
                        dst[:, mt, cols], ps[:], b_sb[:, mt:mt + 1]
                    )

            for tt in range(CHUNK // 128):
                t_tile = n * (CHUNK // 128) + tt
                ps = ps_mm.tile([128, DG], F32, tag="mm", name=f"pv{n}_{tt}")
                for ko in range(KO):
                    nc.tensor.matmul(
                        ps[:],
                        lhsT=xt[:, ko, tt * 128:(tt + 1) * 128],
                        rhs=wv_sb[:, ko, :],
                        start=(ko == 0),
                        stop=(ko == KO - 1),
                    )
                nc.vector.tensor_copy(v_sb[:, t_tile, :, :DH], ps[:])

        def attention(n, mid=None):
            n_m = 4 * (n + 1)          # causal: key tiles 0 .. 4n+3
            # raw (unnormalized) yT and the denominators, evicted promptly
            # from PSUM so the next pair/chunk can reuse the banks
            yT_raw = small.tile([128, 2, CHUNK], F32, tag="ytr", name=f"ytr{n}")
            l_sb = small.tile([128, CHUNK], F32, tag="lsb", name=f"lsb{n}")

            for p in range(2):
                psy = [
                    ps_y.tile([128, CHUNK], F32, tag="y", name=f"psy{n}_{p}_{i}")
                    for i in range(2)
                ]
                for m in range(n_m):
                    qlo = max(0, 128 * m - CHUNK * n)      # first live column
                    lo = min(qlo, CHUNK - 256)             # keep matmul N >= 256
                    diag = m >= 4 * n

                    pss_t = ps_s.tile(
                        [128, 2, CHUNK], F32, tag="s", name=f"s{n}_{m}_{p}"
                    )
                    for half in range(2):
                        rows = slice(64 * half, 64 * half + 64)
                        nc.tensor.matmul(
                            pss_t[:, half, lo:],
                            lhsT=kT_sb[rows, p, m * 128:(m + 1) * 128],
                            rhs=qT_sb[rows, p, n * CHUNK + lo:(n + 1) * CHUNK],
                            start=True,
                            stop=not diag,
                        )
                    if diag:
                        for half in range(2):
                            if qlo > lo:
                                nc.tensor.matmul(
                                    pss_t[:, half, lo:],
                                    lhsT=ident[:],
                                    rhs=wmask[:, 384 - (CHUNK - lo):],
                                    start=False,
                                    stop=True,
                                )
                            else:
                                nc.tensor.matmul(
                                    pss_t[:, half, qlo:qlo + 128],
                                    lhsT=ident[:],
                                    rhs=wmask[:, 256:384],
                                    start=False,
                                    stop=True,
                                )

                    e = expst_pool.tile([128, 2, CHUNK], F32R, tag="e")
                    nc.scalar.activation(
                        e[:, :, lo:], pss_t[:, :, lo:],
                        mybir.ActivationFunctionType.Exp, scale=scale,
                    )

                    for half in range(2):
                        h = 2 * p + half
                        nc.tensor.matmul(
                            psy[half][0:DH + 1, lo:],
                            lhsT=v_sb[:, m, h, :],
                            rhs=e[:, half, lo:],
                            start=(m == 0),
                            stop=(m == n_m - 1),
                        )

                for half in range(2):
                    h = 2 * p + half
                    nc.vector.tensor_copy(
                        yT_raw[64 * half:64 * half + 64, p, :], psy[half][0:DH, :]
                    )
                    nc.vector.tensor_copy(
                        l_sb[32 * h:32 * h + 1, :], psy[half][DH:DH + 1, :]
                    )
                if p == 0 and mid is not None:
                    mid(yT_raw, l_sb)
            return yT_raw, l_sb

        def normalize(n, yT_raw, l_sb):
            recip = small.tile([128, CHUNK], F32, tag="recip", name=f"rc{n}")
            for h in range(HPG):
                nc.vector.reciprocal(
                    recip[32 * h:32 * h + 1, :], l_sb[32 * h:32 * h + 1, :]
                )
            # partition-broadcast of recip rows via a DRAM round-trip (the
            # only 0-step-partition DMA this toolchain accepts)
            recip_dr = dram_pool.tile([HPG, CHUNK], F32, tag="rdr")
            for h in range(HPG):
                nc.sync.dma_start(
                    recip_dr[h:h + 1, :], recip[32 * h:32 * h + 1, :]
                )
            yT_n = small.tile([128, 2, CHUNK], F32R, tag="yt", name=f"yn{n}")
            for ks in range(2):
                bc = small.tile([128, CHUNK], F32, tag="bc", name=f"bc{n}_{ks}")
                for half in range(2):
                    h = 2 * ks + half
                    nc.sync.dma_start(
                        bc[64 * half:64 * half + 64, :],
                        recip_dr[h:h + 1, :].to_broadcast((64, CHUNK)),
                    )
                for half in range(2):
                    nc.vector.tensor_mul(
                        yT_n[64 * half:64 * half + 64, ks, :],
                        yT_raw[64 * half:64 * half + 64, ks, :],
                        bc[64 * half:64 * half + 64, :],
                    )
            return yT_n

        def normalize_fast_ks(n, yT_raw, l_sb, yT_n, rfast, ks):
            """Epilogue variant: broadcast 1/l via a K=1 PE outer-product into
            PSUM instead of the DRAM round-trip, to shorten the tail. One ks
            (head pair) at a time so pair 0 can run mid-attention."""
            with nc.allow_low_precision(reason="f32r is 4-byte fp32 storage"):
                for half in range(2):
                    h = 2 * ks + half
                    nc.vector.reciprocal(
                        rfast[32 * h:32 * h + 1, :], l_sb[32 * h:32 * h + 1, :]
                    )
            bc_ps = [
                ps_mm.tile([128, 512], F32, tag="mm", name=f"bcp{n}_{ks}_{i}")
                for i in range(2)
            ]
            for half in range(2):
                h = 2 * ks + half
                nc.tensor.matmul(
                    bc_ps[half][0:64, :],
                    lhsT=ones_sb[32 * h:32 * h + 1, :],
                    rhs=rfast[32 * h:32 * h + 1, :],
                    start=True,
                    stop=True,
                    tile_position=(32 * h, 0),
                )
            for half in range(2):
                nc.vector.tensor_mul(
                    yT_n[64 * half:64 * half + 64, ks, :],
                    yT_raw[64 * half:64 * half + 64, ks, :],
                    bc_ps[half][0:64, :],
                )

        def outproj(n, yT_n):
            for tt in range(CHUNK // 128):
                t_tile = n * (CHUNK // 128) + tt
                o_sb = out_pool.tile([128, C], F32, tag="o", name=f"o{n}_{tt}")
                for nh in range(2):
                    ps = ps_mm.tile([128, 512], F32, tag="mm", name=f"po{n}_{tt}_{nh}")
                    for ks in range(MT):
                        nc.tensor.matmul(
                            ps[:],
                            lhsT=yT_n[:, ks, tt * 128:(tt + 1) * 128],
                            rhs=wp_box[0][:, ks, nh * 512:(nh + 1) * 512],
                            start=(ks == 0),
                            stop=(ks == MT - 1),
                        )
                    nc.any.tensor_copy(o_sb[:, nh * 512:(nh + 1) * 512], ps[:])
                nc.sync.dma_start(out_r[t_tile], o_sb[:])

        # software pipeline: normalize+outproj for chunk n-1 are emitted
        # after attention(n) so the PE stream never stalls on the
        # normalization round-trip latency
        wp_box = []

        def load_wp():
            wp_sb = const.tile([128, MT, C], F32R)
            nc.sync.dma_start(wp_sb[:], wp.rearrange("(mt p) c -> p mt c", p=128))
            wp_box.append(wp_sb)

        pending = None
        last = NCHUNK - 1
        yn_last = small.tile([128, 2, CHUNK], F32R, tag="yt", name="ynlast")
        rf_last = onebuf.tile([128, CHUNK], F32R, tag="rfast", name="rflast")
        for n in range(NCHUNK):
            proj(n)
            if n == 0:
                load_wp()
            normed = []
            if pending is not None:
                pn, (yr, ls) = pending

                def mid(cur_yr, cur_ls, pn=pn, yr=yr, ls=ls, is_last=(n == last)):
                    normed.append(normalize(pn, yr, ls))
                    if is_last:
                        # pair 0 of the final chunk is done: normalize its
                        # half now so only pair 1's half trails the loop
                        normalize_fast_ks(last, cur_yr, cur_ls, yn_last,
                                          rf_last, 0)
            else:
                mid = None
            state = attention(n, mid=mid)
            if pending is not None:
                outproj(pn, normed[0])
            pending = (n, state)
        pn, (yr, ls) = pending
        normalize_fast_ks(pn, yr, ls, yn_last, rf_last, 1)
        outproj(pn, yn_last)

    return nc


_NC_CACHE = None


def kernel(**inputs) -> np.ndarray:
    global _NC_CACHE
    x = np.asarray(inputs["x"], np.float32)
    Wq = np.asarray(inputs["Wq"], np.float32)
    Wk = np.asarray(inputs["Wk"], np.float32)
    Wv = np.asarray(inputs["Wv"], np.float32)
    Wp = np.asarray(inputs["Wp"], np.float32)
    bq = np.asarray(inputs["bq"], np.float32)
    bk = np.asarray(inputs["bk"], np.float32)
    bv = np.asarray(inputs["bv"], np.float32)
    bp = np.asarray(inputs["bp"], np.float32)

    if _NC_CACHE is None:
        _NC_CACHE = build_kernel()
    nc = _NC_CACHE

    in_maps = []
    for c in range(NCORES):
        b, g = divmod(c, GROUPS)
        rows = slice(g * DG, (g + 1) * DG)
        in_maps.append({
            "xT": np.ascontiguousarray(x[b].T),
            "wq": np.ascontiguousarray(Wq[rows, :].T),
            "wk": np.ascontiguousarray(Wk[rows, :].T),
            "wv": np.ascontiguousarray(Wv[rows, :].T),
            "wp": np.ascontiguousarray(Wp[:, rows].T),
            "bq": np.ascontiguousarray(bq[rows]),
            "bk": np.ascontiguousarray(bk[rows]),
        })

    res = run_bass_kernel_spmd(nc, in_maps, core_ids=list(range(NCORES)))

    result = np.zeros((B, T, C), np.float32)
    for c in range(NCORES):
        b = c // GROUPS
        result[b] += res.results[c]["out"]
    result += (bv @ Wp.T + bp)[None, None, :]
    return result

